# revision 1
# baseline (speedup 1.0000x reference)
"""CP-decomposed conv (1x1 -> depthwise-h -> depthwise-w -> 1x1) on 8 TRN2
NeuronCores, data-parallel over batch.

Per-core pipeline (4 images each):
  stage A: u[r, h', w] = sum_{a,c} x[c, h'+a, w] * W1[(a,c), r]
           (h-depthwise folded into the channel-mixing matmul via
            host-precomputed W1 = f1 (x) f3; bf16 matmuls, fp32 PSUM accum)
  w-taps:  z[r, h', w'] = sum_cw u[r, h', w'+cw] * f2[cw, r]
           (3 taps: ACT tap0 + 2x DVE scalar_tensor_tensor, read PSUM direct)
  stage B: out[f, h', w'] = sum_r f0[f, r] * z[r, h', w']
           (float32r matmul: full PE rate at ~tf32 precision)
Inputs are cast to bf16 on host for x / W1 (halves HBM traffic); output
is stored bf16 and upcast on host.
"""

import numpy as np

B, C, H, W = 32, 256, 128, 128
FH, FW = 3, 3
F, R = 256, 128
HP, WP = H - FH + 1, W - FW + 1  # 126, 126
NCORES = 8
BL = B // NCORES  # images per core

# 126 output rows per image: 10 bands of 12 + 1 band of 6.
BANDS = [(i * 12, 12) for i in range(10)] + [(120, 6)]


def _chunks(bh):
    # (row_offset, nrows) chunks within a band; 4-row chunks (N=512/504).
    out = []
    r0 = 0
    while r0 < bh:
        nr = min(4, bh - r0)
        out.append((r0, nr))
        r0 += nr
    return out


_NC_CACHE = {}


def _build_nc():
    import concourse.bacc as bacc
    import concourse.mybir as mybir
    import concourse.tile as tile

    dt = mybir.dt
    bf16 = dt.bfloat16
    f32 = dt.float32
    f32r = dt.float32r
    mult = mybir.AluOpType.mult
    add = mybir.AluOpType.add

    nc = bacc.Bacc("TRN2", target_bir_lowering=False, debug=False,
                   num_devices=NCORES)

    x_d = nc.dram_tensor("x", [BL, C, H, W], bf16, kind="ExternalInput").ap()
    w1_d = nc.dram_tensor("w1", [FH * C, R], bf16, kind="ExternalInput").ap()
    f0t_d = nc.dram_tensor("f0t", [R, F], f32, kind="ExternalInput").ap()
    f2t_d = nc.dram_tensor("f2t", [R, FW], f32, kind="ExternalInput").ap()
    out_d = nc.dram_tensor("out", [BL, F, HP, WP], bf16,
                           kind="ExternalOutput").ap()

    with tile.TileContext(nc, trace_sim=False) as tc:
        with tc.tile_pool(name="wp", bufs=1) as wp, \
             tc.tile_pool(name="xp", bufs=2) as xp, \
             tc.tile_pool(name="zp", bufs=2) as zp, \
             tc.tile_pool(name="op", bufs=2) as op, \
             tc.tile_pool(name="ups", bufs=2, space="PSUM") as upsp, \
             tc.tile_pool(name="ops", bufs=2, space="PSUM") as opsp:

            # --- weights (resident) ---
            w1_t = wp.tile([128, FH * 2, 128], bf16)  # [c_sub, kt=a*2+ct, r]
            nc.sync.dma_start(
                w1_t[:, :, :],
                w1_d.rearrange("(kt p) r -> p kt r", p=128),
            )
            f2t_t = wp.tile([128, FW], f32)
            nc.sync.dma_start(f2t_t[:, :], f2t_d)
            f0s_t = wp.tile([128, F], f32)
            nc.sync.dma_start(f0s_t[:, :], f0t_d)
            f0r_t = wp.tile([128, F], f32r)
            nc.vector.tensor_copy(f0r_t[:, :], f0s_t[:, :])

            for img in range(BL):
                # --- load x image: [c_sub, ct, h*w] ---
                x_t = xp.tile([128, 2, H * W], bf16, tag="x")
                for ct in range(2):
                    for q in range(4):
                        nc.sync.dma_start(
                            x_t[:, ct, q * 4096:(q + 1) * 4096],
                            x_d[img, ct * 128:(ct + 1) * 128,
                                q * 32:(q + 1) * 32, :],
                        )

                for (h0, bh) in BANDS:
                    # --- stage A: u in PSUM [r, bh*128] ---
                    u_ps = upsp.tile([128, 12 * 128], f32, tag="u")
                    for (r0, nr) in _chunks(bh):
                        n = nr * 128
                        for a in range(FH):
                            for ct in range(2):
                                row = h0 + r0 + a
                                nc.tensor.matmul(
                                    u_ps[:, r0 * 128: r0 * 128 + n],
                                    w1_t[:, a * 2 + ct, :],
                                    x_t[:, ct, row * 128: row * 128 + n],
                                    start=(a == 0 and ct == 0),
                                    stop=(a == FH - 1 and ct == 1),
                                )

                    # --- w-depthwise taps: z = sum_cw u[:, :, cw:cw+126]*f2 ---
                    z_t = zp.tile([128, 12 * WP], f32r, tag="z")
                    zv = z_t[:, 0: bh * WP].rearrange(
                        "p (h w) -> p h w", w=WP)
                    uvf = u_ps[:, 0: bh * 128].rearrange(
                        "p (h w) -> p h w", w=128)
                    nc.scalar.mul(zv, uvf[:, :, 0:WP], f2t_t[:, 0:1])
                    for cw in range(1, FW):
                        nc.vector.scalar_tensor_tensor(
                            zv, uvf[:, :, cw:cw + WP], f2t_t[:, cw:cw + 1],
                            zv, op0=mult, op1=add,
                        )

                    # --- stage B + PSUM->SBUF copy ---
                    o_t = op.tile([128, 2, 12 * WP], bf16, tag="o")
                    ci = 0
                    for (r0, nr) in _chunks(bh):
                        n = nr * WP
                        for ft in range(2):
                            o_ps = opsp.tile([128, 504], f32, tag="ops")
                            nc.tensor.matmul(
                                o_ps[:, 0:n],
                                f0r_t[:, ft * 128:(ft + 1) * 128],
                                z_t[:, r0 * WP: r0 * WP + n],
                                start=True, stop=True,
                            )
                            dst = o_t[:, ft, r0 * WP: r0 * WP + n]
                            if ci % 2 == 0:
                                nc.scalar.copy(dst, o_ps[:, 0:n])
                            else:
                                nc.vector.tensor_copy(dst, o_ps[:, 0:n])
                            ci += 1

                    # --- store band ---
                    for ft in range(2):
                        nc.sync.dma_start(
                            out_d[img, ft * 128:(ft + 1) * 128,
                                  h0:h0 + bh, :],
                            o_t[:, ft, 0: bh * WP],
                        )

    nc.compile()
    return nc


def _get_nc():
    if "nc" not in _NC_CACHE:
        _NC_CACHE["nc"] = _build_nc()
    return _NC_CACHE["nc"]


def _prep_in_maps(x, f0, f1, f2, f3):
    import ml_dtypes
    bf16 = ml_dtypes.bfloat16

    # W1[(a, c), r] = f1[a, r] * f3[c, r]
    w1 = (np.asarray(f1, np.float32)[:, None, :]
          * np.asarray(f3, np.float32)[None, :, :]).reshape(FH * C, R)
    w1b = np.ascontiguousarray(w1.astype(bf16))
    f0t = np.ascontiguousarray(np.asarray(f0, np.float32).T)
    f2t = np.ascontiguousarray(np.asarray(f2, np.float32).T)
    xb = np.ascontiguousarray(np.asarray(x).astype(bf16))
    return [
        {"x": xb[i * BL:(i + 1) * BL], "w1": w1b, "f0t": f0t, "f2t": f2t}
        for i in range(NCORES)
    ]


def kernel(x, f0, f1, f2, f3):
    from concourse import bass_utils

    nc = _get_nc()
    in_maps = _prep_in_maps(x, f0, f1, f2, f3)
    res = bass_utils.run_bass_kernel_spmd(
        nc, in_maps, core_ids=list(range(NCORES)))
    out = np.concatenate(
        [r["out"].astype(np.float32) for r in res.results], axis=0)
    return out


# revision 6
# speedup vs baseline: 1.0781x; 1.0781x over previous
"""CP-decomposed conv (1x1 -> depthwise-h -> depthwise-w -> 1x1) on 8 TRN2
NeuronCores, data-parallel over batch.

Per-core pipeline (4 images each):
  stage A: u[r, h', w] = sum_{a,c} x[c, h'+a, w] * W1[(a,c), r]
           (h-depthwise folded into the channel-mixing matmul via
            host-precomputed W1 = f1 (x) f3; bf16 matmuls, fp32 PSUM accum)
  w-taps:  z[r, h', w'] = sum_cw u[r, h', w'+cw] * f2[cw, r]
           (3 taps: ACT tap0 + 2x DVE scalar_tensor_tensor, read PSUM direct)
  stage B: out[f, h', w'] = sum_r f0[f, r] * z[r, h', w']
           (float32r matmul: full PE rate at ~tf32 precision)
Inputs are cast to bf16 on host for x / W1 (halves HBM traffic); output
is stored bf16 and upcast on host.
"""

import numpy as np

B, C, H, W = 32, 256, 128, 128
FH, FW = 3, 3
F, R = 256, 128
HP, WP = H - FH + 1, W - FW + 1  # 126, 126
NCORES = 8
BL = B // NCORES  # images per core

# 126 output rows per image: 10 bands of 12 + 1 band of 6.
BANDS = [(i * 12, 12) for i in range(10)] + [(120, 6)]


def _chunks(bh):
    # (row_offset, nrows) chunks within a band; 4-row chunks (N=512/504).
    out = []
    r0 = 0
    while r0 < bh:
        nr = min(4, bh - r0)
        out.append((r0, nr))
        r0 += nr
    return out


_NC_CACHE = {}


def _build_nc():
    import concourse.bacc as bacc
    import concourse.mybir as mybir
    import concourse.tile as tile

    dt = mybir.dt
    bf16 = dt.bfloat16
    f32 = dt.float32
    f32r = dt.float32r
    mult = mybir.AluOpType.mult
    add = mybir.AluOpType.add

    nc = bacc.Bacc("TRN2", target_bir_lowering=False, debug=False,
                   num_devices=NCORES)

    x_d = nc.dram_tensor("x", [BL, C, H, W], bf16, kind="ExternalInput").ap()
    w1_d = nc.dram_tensor("w1", [FH * C, R], bf16, kind="ExternalInput").ap()
    f0t_d = nc.dram_tensor("f0t", [R, F], f32, kind="ExternalInput").ap()
    f2t_d = nc.dram_tensor("f2t", [R, FW], f32, kind="ExternalInput").ap()
    out_d = nc.dram_tensor("out", [BL, F, HP, WP], bf16,
                           kind="ExternalOutput").ap()

    with tile.TileContext(nc, trace_sim=False) as tc:
        with tc.tile_pool(name="wp", bufs=1) as wp, \
             tc.tile_pool(name="xp", bufs=2) as xp, \
             tc.tile_pool(name="zp", bufs=3) as zp, \
             tc.tile_pool(name="op", bufs=3) as op, \
             tc.tile_pool(name="ups", bufs=2, space="PSUM") as upsp, \
             tc.tile_pool(name="ops", bufs=2, space="PSUM") as opsp:

            # --- weights (resident) ---
            w1_t = wp.tile([128, FH * 2, 128], bf16)  # [c_sub, kt=a*2+ct, r]
            nc.sync.dma_start(
                w1_t[:, :, :],
                w1_d.rearrange("(kt p) r -> p kt r", p=128),
            )
            f2t_t = wp.tile([128, FW], f32)
            nc.sync.dma_start(f2t_t[:, :], f2t_d)
            f0s_t = wp.tile([128, F], f32)
            nc.sync.dma_start(f0s_t[:, :], f0t_d)
            f0r_t = wp.tile([128, F], f32r)
            nc.vector.tensor_copy(f0r_t[:, :], f0s_t[:, :])

            for img in range(BL):
                # --- load x image: [c_sub, ct, h*w] ---
                x_t = xp.tile([128, 2, H * W], bf16, tag="x")
                for q in range(8):
                    for ct in range(2):
                        nc.sync.dma_start(
                            x_t[:, ct, q * 2048:(q + 1) * 2048],
                            x_d[img, ct * 128:(ct + 1) * 128,
                                q * 16:(q + 1) * 16, :],
                        )

                for (h0, bh) in BANDS:
                    # --- stage A: u in PSUM [r, bh*128] ---
                    # weight-outer order: consecutive matmuls share lhsT so
                    # walrus ldw-opt elides the redundant LDWEIGHTS.
                    u_ps = upsp.tile([128, 12 * 128], f32, tag="u")
                    for a in range(FH):
                        for ct in range(2):
                            for (r0, nr) in _chunks(bh):
                                n = nr * 128
                                row = h0 + r0 + a
                                nc.tensor.matmul(
                                    u_ps[:, r0 * 128: r0 * 128 + n],
                                    w1_t[:, a * 2 + ct, :],
                                    x_t[:, ct, row * 128: row * 128 + n],
                                    start=(a == 0 and ct == 0),
                                    stop=(a == FH - 1 and ct == 1),
                                )

                    # --- w-depthwise taps: z = sum_cw u[:, :, cw:cw+126]*f2 ---
                    z_t = zp.tile([128, 12 * WP], f32r, tag="z")
                    zv = z_t[:, 0: bh * WP].rearrange(
                        "p (h w) -> p h w", w=WP)
                    uvf = u_ps[:, 0: bh * 128].rearrange(
                        "p (h w) -> p h w", w=128)
                    nc.scalar.mul(zv, uvf[:, :, 0:WP], f2t_t[:, 0:1])
                    for cw in range(1, FW):
                        nc.vector.scalar_tensor_tensor(
                            zv, uvf[:, :, cw:cw + WP], f2t_t[:, cw:cw + 1],
                            zv, op0=mult, op1=add,
                        )

                    # --- stage B + PSUM->SBUF copy ---
                    o_t = op.tile([128, 2, 12 * WP], bf16, tag="o")
                    ci = 0
                    for ft in range(2):
                        for (r0, nr) in _chunks(bh):
                            n = nr * WP
                            o_ps = opsp.tile([128, 504], f32, tag="ops")
                            nc.tensor.matmul(
                                o_ps[:, 0:n],
                                f0r_t[:, ft * 128:(ft + 1) * 128],
                                z_t[:, r0 * WP: r0 * WP + n],
                                start=True, stop=True,
                            )
                            dst = o_t[:, ft, r0 * WP: r0 * WP + n]
                            if ci % 4 == 3:
                                nc.vector.tensor_copy(dst, o_ps[:, 0:n])
                            else:
                                nc.scalar.copy(dst, o_ps[:, 0:n])
                            ci += 1

                    # --- store band ---
                    for ft in range(2):
                        nc.sync.dma_start(
                            out_d[img, ft * 128:(ft + 1) * 128,
                                  h0:h0 + bh, :],
                            o_t[:, ft, 0: bh * WP],
                        )

    nc.compile()
    return nc


def _get_nc():
    if "nc" not in _NC_CACHE:
        _NC_CACHE["nc"] = _build_nc()
    return _NC_CACHE["nc"]


def _prep_in_maps(x, f0, f1, f2, f3):
    import ml_dtypes
    bf16 = ml_dtypes.bfloat16

    # W1[(a, c), r] = f1[a, r] * f3[c, r]
    w1 = (np.asarray(f1, np.float32)[:, None, :]
          * np.asarray(f3, np.float32)[None, :, :]).reshape(FH * C, R)
    w1b = np.ascontiguousarray(w1.astype(bf16))
    f0t = np.ascontiguousarray(np.asarray(f0, np.float32).T)
    f2t = np.ascontiguousarray(np.asarray(f2, np.float32).T)
    xb = np.ascontiguousarray(np.asarray(x).astype(bf16))
    return [
        {"x": xb[i * BL:(i + 1) * BL], "w1": w1b, "f0t": f0t, "f2t": f2t}
        for i in range(NCORES)
    ]


def kernel(x, f0, f1, f2, f3):
    from concourse import bass_utils

    nc = _get_nc()
    in_maps = _prep_in_maps(x, f0, f1, f2, f3)
    res = bass_utils.run_bass_kernel_spmd(
        nc, in_maps, core_ids=list(range(NCORES)))
    out = np.concatenate(
        [r["out"].astype(np.float32) for r in res.results], axis=0)
    return out


# revision 7
# speedup vs baseline: 1.2387x; 1.1490x over previous
"""CP-decomposed conv (1x1 -> depthwise-h -> depthwise-w -> 1x1) on 8 TRN2
NeuronCores, data-parallel over batch.

Per-core pipeline (4 images each):
  stage A: u[r, h', w] = sum_{a,c} x[c, h'+a, w] * W1[(a,c), r]
           (h-depthwise folded into the channel-mixing matmul via
            host-precomputed W1 = f1 (x) f3; bf16 matmuls, fp32 PSUM accum)
  w-taps:  z[r, h', w'] = sum_cw u[r, h', w'+cw] * f2[cw, r], computed as
             zext = u * f2[0]          (fused into the PSUM->SBUF copy, ACT)
             z = zext[+1]*r1 + zext[+0]  (DVE)   r_c = f2[cw]/f2[0]
             z = zext[+2]*r2 + z         (DVE)
  stage B: out[f, h', w'] = sum_r f0[f, r] * z[r, h', w']  (bf16 matmul)
x / W1 / f0 are cast to bf16 on host; output is stored bf16 and upcast.
"""

import numpy as np

B, C, H, W = 32, 256, 128, 128
FH, FW = 3, 3
F, R = 256, 128
HP, WP = H - FH + 1, W - FW + 1  # 126, 126
NCORES = 8
BL = B // NCORES  # images per core

# 126 output rows per image: 15 bands of 8 + 1 band of 6.
BANDS = [(i * 8, 8) for i in range(15)] + [(120, 6)]


def _chunks(bh):
    out = []
    r0 = 0
    while r0 < bh:
        nr = min(4, bh - r0)
        out.append((r0, nr))
        r0 += nr
    return out


_NC_CACHE = {}


def _build_nc():
    import concourse.bacc as bacc
    import concourse.mybir as mybir
    import concourse.tile as tile

    dt = mybir.dt
    bf16 = dt.bfloat16
    f32 = dt.float32
    mult = mybir.AluOpType.mult
    add = mybir.AluOpType.add

    nc = bacc.Bacc("TRN2", target_bir_lowering=False, debug=False,
                   num_devices=NCORES)

    x_d = nc.dram_tensor("x", [BL, C, H, W], bf16, kind="ExternalInput").ap()
    w1_d = nc.dram_tensor("w1", [FH * C, R], bf16, kind="ExternalInput").ap()
    f0t_d = nc.dram_tensor("f0t", [R, F], bf16, kind="ExternalInput").ap()
    # f2s[r] = [f2[0,r], f2[1,r]/f2[0,r], f2[2,r]/f2[0,r]]
    f2s_d = nc.dram_tensor("f2s", [R, FW], f32, kind="ExternalInput").ap()
    out_d = nc.dram_tensor("out", [BL, F, HP, WP], bf16,
                           kind="ExternalOutput").ap()

    with tile.TileContext(nc, trace_sim=False) as tc:
        with tc.tile_pool(name="wp", bufs=1) as wp, \
             tc.tile_pool(name="xp", bufs=2) as xp, \
             tc.tile_pool(name="ep", bufs=3) as ep, \
             tc.tile_pool(name="zp", bufs=3) as zp, \
             tc.tile_pool(name="op", bufs=3) as op, \
             tc.tile_pool(name="ups", bufs=3, space="PSUM") as upsp, \
             tc.tile_pool(name="ops", bufs=2, space="PSUM") as opsp:

            # --- weights (resident) ---
            w1_t = wp.tile([128, FH * 2, 128], bf16)  # [c_sub, kt=a*2+ct, r]
            nc.sync.dma_start(
                w1_t[:, :, :],
                w1_d.rearrange("(kt p) r -> p kt r", p=128),
            )
            f2s_t = wp.tile([128, FW], f32)
            nc.sync.dma_start(f2s_t[:, :], f2s_d)
            f0t_t = wp.tile([128, F], bf16)
            nc.sync.dma_start(f0t_t[:, :], f0t_d)

            ci = 0
            for img in range(BL):
                # --- load x image: [c_sub, ct, h*w] ---
                x_t = xp.tile([128, 2, H * W], bf16, tag="x")
                for q in range(8):
                    for ct in range(2):
                        nc.sync.dma_start(
                            x_t[:, ct, q * 2048:(q + 1) * 2048],
                            x_d[img, ct * 128:(ct + 1) * 128,
                                q * 16:(q + 1) * 16, :],
                        )

                for (h0, bh) in BANDS:
                    # --- stage A: u in PSUM [r, bh*128]; weight-outer so
                    # consecutive matmuls keep the same stationary tile ---
                    u_ps = upsp.tile([128, 8 * 128], f32, tag="u")
                    for a in range(FH):
                        for ct in range(2):
                            for (r0, nr) in _chunks(bh):
                                n = nr * 128
                                row = h0 + r0 + a
                                nc.tensor.matmul(
                                    u_ps[:, r0 * 128: r0 * 128 + n],
                                    w1_t[:, a * 2 + ct, :],
                                    x_t[:, ct, row * 128: row * 128 + n],
                                    start=(a == 0 and ct == 0),
                                    stop=(a == FH - 1 and ct == 1),
                                )

                    # --- fused PSUM->SBUF copy * f2[0] (tap0), then taps ---
                    ze_t = ep.tile([128, 8 * 128], bf16, tag="ze")
                    nc.scalar.mul(ze_t[:, 0:bh * 128], u_ps[:, 0:bh * 128],
                                  f2s_t[:, 0:1])
                    z_t = zp.tile([128, 8 * WP], bf16, tag="z")
                    zv = z_t[:, 0:bh * WP].rearrange("p (h w) -> p h w", w=WP)
                    zev = ze_t[:, 0:bh * 128].rearrange(
                        "p (h w) -> p h w", w=128)
                    nc.vector.scalar_tensor_tensor(
                        zv, zev[:, :, 1:1 + WP], f2s_t[:, 1:2],
                        zev[:, :, 0:WP], op0=mult, op1=add)
                    nc.vector.scalar_tensor_tensor(
                        zv, zev[:, :, 2:2 + WP], f2s_t[:, 2:3],
                        zv, op0=mult, op1=add)

                    # --- stage B (bf16) + PSUM->SBUF copy ---
                    o_t = op.tile([128, 2, 8 * WP], bf16, tag="o")
                    for ft in range(2):
                        for (r0, nr) in _chunks(bh):
                            n = nr * WP
                            o_ps = opsp.tile([128, 504], f32, tag="ops")
                            nc.tensor.matmul(
                                o_ps[:, 0:n],
                                f0t_t[:, ft * 128:(ft + 1) * 128],
                                z_t[:, r0 * WP: r0 * WP + n],
                                start=True, stop=True,
                            )
                            dst = o_t[:, ft, r0 * WP: r0 * WP + n]
                            if ci % 3 == 2:
                                nc.vector.tensor_copy(dst, o_ps[:, 0:n])
                            else:
                                nc.scalar.copy(dst, o_ps[:, 0:n])
                            ci += 1

                    # --- store band ---
                    for ft in range(2):
                        nc.sync.dma_start(
                            out_d[img, ft * 128:(ft + 1) * 128,
                                  h0:h0 + bh, :],
                            o_t[:, ft, 0: bh * WP],
                        )

    nc.compile()
    return nc


def _get_nc():
    if "nc" not in _NC_CACHE:
        _NC_CACHE["nc"] = _build_nc()
    return _NC_CACHE["nc"]


def _prep_in_maps(x, f0, f1, f2, f3):
    import ml_dtypes
    bf16 = ml_dtypes.bfloat16

    # W1[(a, c), r] = f1[a, r] * f3[c, r]
    w1 = (np.asarray(f1, np.float32)[:, None, :]
          * np.asarray(f3, np.float32)[None, :, :]).reshape(FH * C, R)
    w1b = np.ascontiguousarray(w1.astype(bf16))
    f0t = np.ascontiguousarray(np.asarray(f0, np.float32).T.astype(bf16))
    f2 = np.asarray(f2, np.float64)
    s0 = f2[0].copy()
    s0[np.abs(s0) < 1e-30] = 1e-30
    f2s = np.stack([s0, f2[1] / s0, f2[2] / s0], axis=1).astype(np.float32)
    f2s = np.ascontiguousarray(f2s)
    xb = np.ascontiguousarray(np.asarray(x).astype(bf16))
    return [
        {"x": xb[i * BL:(i + 1) * BL], "w1": w1b, "f0t": f0t, "f2s": f2s}
        for i in range(NCORES)
    ]


def kernel(x, f0, f1, f2, f3):
    from concourse import bass_utils

    nc = _get_nc()
    in_maps = _prep_in_maps(x, f0, f1, f2, f3)
    res = bass_utils.run_bass_kernel_spmd(
        nc, in_maps, core_ids=list(range(NCORES)))
    out = np.concatenate(
        [r["out"].astype(np.float32) for r in res.results], axis=0)
    return out
